# revision 43
# baseline (speedup 1.0000x reference)
"""Accurate SDF (garment-to-body signed distance) on 8 Trainium2 cores — v7.

Faces sharded 8 ways (1722/core, padded to 14*128). Device score per
(face f, point g):

    S = (K*h)^2 + relu(max(K*s_ab, K*s_ca, K*s_bc))^2      (K = 128)

where h is the signed plane distance and s_e are in-plane signed
distances to the three edge lines (positive outside). S equals K^2 * d^2
exactly in face/edge regions and under-estimates by at most 2x in vertex
regions — enough for the host to take the top-M faces per point by S and
exactly re-rank them (fp64) with the reference formulas. Degenerate
faces score +inf on device and are appended to every point's candidate
list on the host.

All four quantities are affine in p, so each is ONE fp32r matmul per
512-point chunk: weights [m_xyz | c_hi | c_lo] against moving rows
[px, py, pz, 1, 1]. Geometry (verts and points) is snapped to fp32r so
weights are exact and the matmul computes the exact score of a ~1e-3
perturbed problem; the c_hi/c_lo split keeps the affine constant to
fp32 accuracy. The K=2^7 scale keeps fp16 outputs away from the
denormal/FTZ zone without overflowing genuine candidates.

Per (b, face-tile) iteration, [128, 1024] ops balanced across engines
(Act / DVE / Pool(gpsimd) ~2.6us each, steady II 2.66us):
  a  = relu(s_ab)            Act   (makes the max-chain >= 0)
  m1 = max(a, s_ca)          DVE   (one PSUM operand per vector op)
  m2 = max(m1, s_bc) -> f16  DVE
  sf = h^2 -> f16            Act
  qm = m2^2 -> f16           Act cols [0,CQ) / Pool f16 mult [CQ,1024)
  S  = qm + sf   f16         DVE cols [0,CSOL) / Pool add [CSOL,1024)
PSUM: 2 names x bufs=2 x 2 banks double-buffer matmuls against drains.
Startup: 3 junk bf16 matmuls ramp the PE p-state while the weights DMA
in three parallel chunks (SP head slice / Pool points / Act bulk).
"""

import numpy as np

B, G, V, F = 2, 1024, 6890, 13776
NCORES = 8
FC = F // NCORES            # 1722 faces per core
FTILES = 14                 # ceil(1722/128)
FPAD = FTILES * 128         # 1792
GCHUNK = 512
NQ = 4                      # quantities: s_ab, s_ca, s_bc, h
NROW = 5                    # moving rows: px, py, pz, 1, 1
PCOLS = B * G               # snapped points packed at the head of wq
WCOLS = B * FTILES * NQ * 128
SC = np.float64(128.0)      # weight scale (exact power of 2)
BIGH = np.float32(1e6)      # h-const for pad/degenerate faces -> S = inf
TOPM = 48                   # host: faces re-ranked exactly per point

_CACHE = {}


def _build_bass():
    import concourse.bass as bass
    import concourse.bacc as bacc
    import concourse.mybir as mybir
    from concourse.tile import TileContext

    dt = mybir.dt.float32
    dtr = mybir.dt.float32r
    dth = mybir.dt.float16
    Alu = mybir.AluOpType
    Act = mybir.ActivationFunctionType

    nc = bacc.Bacc()

    wq_d = nc.declare_dram_parameter("wq", [NROW, PCOLS + WCOLS], dtr,
                                     isOutput=False)
    outs_d = nc.declare_dram_parameter("out_s", [B, FTILES, 128, G], dth,
                                       isOutput=True)

    Vv = nc.vector
    Gg = nc.gpsimd
    Ss = nc.scalar
    Tt = nc.tensor
    Sy = nc.sync

    GW = 2 * GCHUNK
    sh = [128, GW]

    with TileContext(nc) as tc:
        with (
            tc.tile_pool(name="cpool", bufs=1) as cpool,
            tc.tile_pool(name="work", bufs=1) as work,
            tc.tile_pool(name="mm", bufs=2, space="PSUM") as mm,
        ):
            wq_s = cpool.tile([NROW, PCOLS + WCOLS], dtr, name="wq_s")
            # One head DMA delivers the points block plus the first three
            # face-tiles' weights so compute starts while the bulk
            # transfers through the Act DGE in parallel.
            HDW = PCOLS + 3 * NQ * 128
            Sy.dma_start(wq_s[:, :HDW], wq_d[:, :HDW])
            Ss.dma_start(wq_s[:, HDW:], wq_d[:, HDW:])
            # Warm up the PE p-state during the DMA wait with junk matmuls
            # on a zeroed stationary tile (results never read).
            dtb = mybir.dt.bfloat16
            wz = cpool.tile([NROW, 128 + GCHUNK], dtb, name="wz")
            Gg.memset(wz[:], 0)
            warm = mm.tile(sh, dt, name="px")
            for _ in range(3):
                Tt.matmul(warm[:, :GCHUNK], wz[:, :128], wz[:, 128:],
                          start=True, stop=True)

            def wt(nm, dtype=dth):
                return work.tile(sh, dtype, name=nm, bufs=4)

            def MM(b, ft, q, name):
                t = mm.tile(sh, dt, name=name)
                col = PCOLS + ((b * FTILES + ft) * NQ + q) * 128
                for h in range(2):
                    g0 = b * G + h * GCHUNK
                    Tt.matmul(t[:, h * GCHUNK:(h + 1) * GCHUNK],
                              wq_s[:, col:col + 128],
                              wq_s[:, g0:g0 + GCHUNK],
                              start=True, stop=True)
                return t

            # S = h^2 + relu(max(s_ab, s_ca, s_bc))^2. a = relu(s_ab)
            # makes the max-chain >= 0 so the square needs no relu.
            # Constraints: GPSIMD (Pool) can't touch PSUM and only does
            # add/mult; vector ops read at most one PSUM operand. Engine
            # balance: Act {a, sf, part of qm}, DVE {m1, m2->f16, part of
            # So}, Pool {rest of qm as f16 mult, rest of So}. PSUM: 2
            # names x bufs=2 (8 banks) double-buffers matmuls vs drains.
            CQ = 464                # qm columns squared by Act
            CSOL = 400              # So columns added by DVE

            def stage2(c):
                """Square + add + DMA for a finished face-tile."""
                b, ft, m2, sf = c
                qm = wt("qm")
                Ss.activation(qm[:, :CQ], m2[:, :CQ], Act.Square)
                Gg.tensor_tensor(qm[:, CQ:], m2[:, CQ:], m2[:, CQ:],
                                 Alu.mult)
                So = wt("So")
                last = (b, ft) == (B - 1, FTILES - 1)
                cso = GW if last else CSOL
                Vv.tensor_tensor(So[:, :cso], qm[:, :cso],
                                 sf[:, :cso], Alu.add)
                if not last:
                    Gg.tensor_tensor(So[:, CSOL:], qm[:, CSOL:],
                                     sf[:, CSOL:], Alu.add)
                if last:
                    # tail: overlap the DMA issue with the Pool add
                    Sy.dma_start(outs_d[b, ft, :, :CSOL], So[:, :CSOL])
                    Sy.dma_start(outs_d[b, ft, :, CSOL:], So[:, CSOL:])
                else:
                    Sy.dma_start(outs_d[b, ft], So[:])

            prev = None
            for b in range(B):
                for ft in range(FTILES):
                    pab = MM(b, ft, 0, "px")
                    pca = MM(b, ft, 1, "py")
                    a = wt("a", dt)
                    Ss.activation(a[:], pab[:], Act.Relu)
                    m1 = wt("m1", dt)
                    Vv.tensor_tensor(m1[:], a[:], pca[:], Alu.max)
                    pbc = MM(b, ft, 2, "px")
                    ph_ = MM(b, ft, 3, "py")
                    m2 = wt("m2")
                    Vv.tensor_tensor(m2[:], m1[:], pbc[:], Alu.max)
                    sf = wt("sf")
                    Ss.activation(sf[:], ph_[:], Act.Square)
                    if prev is not None:
                        stage2(prev)
                    prev = (b, ft, m2, sf)
            stage2(prev)
    nc.finalize()
    return nc


def _get_nc():
    if "nc" not in _CACHE:
        _CACHE["nc"] = _build_bass()
    return _CACHE["nc"]


def _round_fp32r(x):
    """Round fp32 -> fp32r container (11-bit mantissa, RNE)."""
    u = np.ascontiguousarray(x, np.float32).view(np.uint32)
    base = u & np.uint32(0xFFFFF000)
    low = u & np.uint32(0x00000FFF)
    half = np.uint32(0x800)
    lsb = (base >> np.uint32(12)) & np.uint32(1)
    up = (low > half) | ((low == half) & (lsb == 1))
    return np.where(up, base + np.uint32(0x1000), base).view(np.float32)


def _face_geom(batch_body_verts, body_faces):
    """Snapped fp64 per-face geometry + degeneracy mask, per batch.

    Returns (a, b, c [B,F,3] fp64 snapped verts, degen [B,F] bool)."""
    f32 = np.float32
    out = []
    for bi in range(B):
        vs = _round_fp32r(batch_body_verts[bi].astype(f32)).astype(np.float64)
        fv = vs[body_faces]
        a, bb, cc = fv[:, 0], fv[:, 1], fv[:, 2]
        ab, ac = bb - a, cc - a
        n = np.cross(ab, ac)
        nn = np.linalg.norm(n, axis=1)
        den = (np.sum(ab * ab, -1) * np.sum(ac * ac, -1)
               - np.sum(ab * ac, -1) ** 2)
        degen = (den < 1e-8) | (nn < 1e-10)
        out.append((a, bb, cc, n, nn, degen))
    return out


def _core_inputs(batch_garment_verts, batch_body_verts, body_faces, geom):
    f32 = np.float32
    gv = batch_garment_verts.astype(f32)
    p_snap = _round_fp32r(gv)                             # [B,G,3]
    p5 = np.zeros((NROW, B * G), f32)
    for bi in range(B):
        p5[0:3, bi * G:(bi + 1) * G] = p_snap[bi].T
    p5[3] = 1.0
    p5[4] = 1.0

    def hi_lo(c):
        chi = _round_fp32r(c.astype(f32)).astype(np.float64)
        clo = _round_fp32r((c - chi).astype(f32))
        return chi.astype(f32), clo

    in_maps = []
    for c in range(NCORES):
        sl = slice(c * FC, (c + 1) * FC)
        wq = np.zeros((NROW, PCOLS + WCOLS), f32)
        wq[:, :PCOLS] = p5
        for bi in range(B):
            a, bb, cc, n, nn, degen = geom[bi]
            a, bb, cc = a[sl], bb[sl], cc[sl]
            n, nn, degen = n[sl], nn[sl], degen[sl]
            nh = n / np.maximum(nn, 1e-30)[:, None]
            rows = np.zeros((NQ, NROW, FPAD), f32)
            for qi, (eu, ev, w) in enumerate(
                    ((a, bb, cc), (cc, a, bb), (bb, cc, a))):
                ed = ev - eu
                m = np.cross(ed, nh)
                m = m / np.maximum(np.linalg.norm(m, axis=1), 1e-30)[:, None]
                flip = np.sum(m * (w - eu), -1) > 0
                m = np.where(flip[:, None], -m, m) * SC
                m_r = _round_fp32r(m.astype(f32)).astype(np.float64)
                chi, clo = hi_lo(-np.sum(m_r * eu, -1))
                rows[qi, 0:3, :FC] = m_r.T.astype(f32)
                rows[qi, 3, :FC] = chi
                rows[qi, 4, :FC] = clo
            nh_r = _round_fp32r((nh * SC).astype(f32)).astype(np.float64)
            chi, clo = hi_lo(-np.sum(nh_r * a, -1))
            rows[3, 0:3, :FC] = nh_r.T.astype(f32)
            rows[3, 3, :FC] = np.where(degen, BIGH, chi)
            rows[3, 4, :FC] = np.where(degen, 0.0, clo)
            rows[3, 0:3, :FC][:, degen] = 0.0
            rows[0:3, :, :FC][:, :, degen] = 0.0
            rows[3, 3, FC:] = BIGH                        # pad faces -> inf
            for ft in range(FTILES):
                fsl = slice(ft * 128, (ft + 1) * 128)
                for q in range(NQ):
                    c0 = PCOLS + ((bi * FTILES + ft) * NQ + q) * 128
                    wq[:, c0:c0 + 128] = rows[q][:, fsl]
        in_maps.append({"wq": wq})
    return in_maps


def _d2_exact64_cand(p, bverts, faces, cand):
    """Exact fp64 point-triangle dist^2 for candidate faces. cand [G,C]."""
    fv = bverts[faces[cand]].astype(np.float64)      # [G,C,3,3]
    a, b, c = fv[:, :, 0], fv[:, :, 1], fv[:, :, 2]
    q = p.astype(np.float64)[:, None, :]
    best = np.full(cand.shape, np.inf)
    for ea, eb in ((a, b), (b, c), (c, a)):
        ed = eb - ea
        L2 = np.sum(ed * ed, -1)
        pe = q - ea
        t = np.clip(np.sum(pe * ed, -1) / np.maximum(L2, 1e-300), 0, 1)
        d = pe - t[..., None] * ed
        best = np.minimum(best, np.sum(d * d, -1))
    ab, ac = b - a, c - a
    n = np.cross(ab, ac)
    naa = np.sum(ab * ab, -1); nab = np.sum(ab * ac, -1)
    ncc = np.sum(ac * ac, -1)
    den = naa * ncc - nab * nab
    pa = q - a
    d1 = np.sum(pa * ab, -1); d2_ = np.sum(pa * ac, -1)
    vb = ncc * d1 - nab * d2_; vc = naa * d2_ - nab * d1
    va = den - vb - vc
    inside = (vb >= 0) & (vc >= 0) & (va >= 0) & (den > 1e-300)
    hn = np.sum(pa * n, -1)
    h2 = hn * hn / np.maximum(den, 1e-300)
    return np.where(inside, np.minimum(best, h2), best)


def _host_finish(g_verts, b_verts, faces, tri):
    """Exact reference finish for the winning face of each garment point."""
    f32 = np.float32
    EPS = f32(1e-10)

    def safe(x):
        return np.where(np.abs(x) < 1e-12, f32(1e-12), x).astype(f32)

    fverts = b_verts[faces]
    a_, b_, c_ = fverts[:, 0], fverts[:, 1], fverts[:, 2]
    fn_raw = np.cross(b_ - a_, c_ - a_).astype(f32)
    vn = np.zeros_like(b_verts)
    for k in range(3):
        np.add.at(vn, faces[:, k], fn_raw)
    vn = vn / (np.linalg.norm(vn, axis=-1, keepdims=True).astype(f32) + EPS)
    fn = fn_raw / (np.linalg.norm(fn_raw, axis=-1, keepdims=True).astype(f32) + EPS)

    a = a_[tri]; bb = b_[tri]; cc = c_[tri]
    q = g_verts
    ab = bb - a; ac = cc - a
    ap = q - a
    d1 = np.sum(ab * ap, -1); d2 = np.sum(ac * ap, -1)
    bp = q - bb
    d3 = np.sum(ab * bp, -1); d4 = np.sum(ac * bp, -1)
    cp = q - cc
    d5 = np.sum(ab * cp, -1); d6 = np.sum(ac * cp, -1)
    vc = d1 * d4 - d3 * d2
    vb = d5 * d2 - d1 * d6
    va = d3 * d6 - d5 * d4
    denom = safe(va + vb + vc)
    v, w = (vb / denom).astype(f32), (vc / denom).astype(f32)
    part = np.zeros(v.shape, np.int32)
    t_bc = ((d4 - d3) / safe((d4 - d3) + (d5 - d6))).astype(f32)
    m = (va <= 0) & (d4 - d3 >= 0) & (d5 - d6 >= 0)
    v = np.where(m, 1.0 - t_bc, v).astype(f32)
    w = np.where(m, t_bc, w).astype(f32)
    part = np.where(m, 2, part)
    t_ac = (d2 / safe(d2 - d6)).astype(f32)
    m = (vb <= 0) & (d2 >= 0) & (d6 <= 0)
    v = np.where(m, 0.0, v).astype(f32)
    w = np.where(m, t_ac, w).astype(f32)
    part = np.where(m, 3, part)
    m = (d6 >= 0) & (d5 <= d6)
    v = np.where(m, 0.0, v).astype(f32)
    w = np.where(m, 1.0, w).astype(f32)
    part = np.where(m, 6, part)
    t_ab = (d1 / safe(d1 - d3)).astype(f32)
    m = (vc <= 0) & (d1 >= 0) & (d3 <= 0)
    v = np.where(m, t_ab, v).astype(f32)
    w = np.where(m, 0.0, w).astype(f32)
    part = np.where(m, 1, part)
    m = (d3 >= 0) & (d4 <= d3)
    v = np.where(m, 1.0, v).astype(f32)
    w = np.where(m, 0.0, w).astype(f32)
    part = np.where(m, 5, part)
    m = (d1 <= 0) & (d2 <= 0)
    v = np.where(m, 0.0, v).astype(f32)
    w = np.where(m, 0.0, w).astype(f32)
    part = np.where(m, 4, part)
    npt = a + v[:, None] * ab + w[:, None] * ac

    fidx = faces[tri]
    gar = np.arange(len(tri))
    take = lambda col: vn[fidx[gar, col]]
    n_face = fn[tri]
    n_vert = take(np.clip(part - 4, 0, 2))
    n_edge = take(np.clip(part - 1, 0, 2)) + take(np.mod(part, 3))
    n = np.where((part == 0)[:, None], n_face,
                 np.where((part > 3)[:, None], n_vert, n_edge)).astype(f32)
    n = n / (np.linalg.norm(n, axis=-1, keepdims=True).astype(f32) + EPS)
    return np.sum((g_verts - npt) * n, axis=1).astype(f32)


def kernel(batch_garment_verts, batch_body_verts, body_faces, _profile=None):
    from concourse.bass_utils import run_bass_kernel_spmd

    batch_garment_verts = np.asarray(batch_garment_verts, dtype=np.float32)
    batch_body_verts = np.asarray(batch_body_verts, dtype=np.float32)
    body_faces = np.asarray(body_faces)

    nc = _get_nc()
    geom = _face_geom(batch_body_verts, body_faces)
    in_maps = _core_inputs(batch_garment_verts, batch_body_verts,
                           body_faces, geom)
    kwargs = dict(_profile) if _profile else {}
    res = run_bass_kernel_spmd(nc, in_maps, list(range(NCORES)), **kwargs)
    if _profile is not None:
        _CACHE["last_results"] = res

    vals = np.stack([np.asarray(r["out_s"]) for r in res.results])
    # [NC,B,FT,128,G] -> [B,G,NC*FPAD]
    flat = vals.astype(np.float32).transpose(1, 4, 0, 2, 3).reshape(
        B, G, NCORES * FPAD)
    local = np.arange(NCORES * FPAD) % FPAD
    flat = np.where(local[None, None, :] < FC, flat, np.inf)
    out = np.empty((B, G), np.float32)
    for b in range(B):
        degen_ids = np.nonzero(geom[b][5])[0]
        top = np.argpartition(flat[b], TOPM, axis=1)[:, :TOPM]  # [G, M]
        cand = (top // FPAD) * FC + (top % FPAD)                # global face id
        if degen_ids.size:
            cand = np.concatenate(
                [cand, np.broadcast_to(degen_ids, (G, degen_ids.size))], 1)
        dref = _d2_exact64_cand(batch_garment_verts[b], batch_body_verts[b],
                                body_faces, cand)
        mn = dref.min(axis=1, keepdims=True)
        sel = np.where(dref == mn, cand, F + 1)
        tri = sel.min(axis=1)
        out[b] = _host_finish(batch_garment_verts[b], batch_body_verts[b],
                              body_faces, tri)
    return out


# revision 44
# speedup vs baseline: 1.0026x; 1.0026x over previous
"""Accurate SDF (garment-to-body signed distance) on 8 Trainium2 cores — v7.

Faces sharded 8 ways (1722/core, padded to 14*128). Device score per
(face f, point g):

    S = (K*h)^2 + relu(max(K*s_ab, K*s_ca, K*s_bc))^2      (K = 128)

where h is the signed plane distance and s_e are in-plane signed
distances to the three edge lines (positive outside). S equals K^2 * d^2
exactly in face/edge regions and under-estimates by at most 2x in vertex
regions — enough for the host to take the top-M faces per point by S and
exactly re-rank them (fp64) with the reference formulas. Degenerate
faces score +inf on device and are appended to every point's candidate
list on the host.

All four quantities are affine in p, so each is ONE fp32r matmul per
512-point chunk: weights [m_xyz | c_hi | c_lo] against moving rows
[px, py, pz, 1, 1]. Geometry (verts and points) is snapped to fp32r so
weights are exact and the matmul computes the exact score of a ~1e-3
perturbed problem; the c_hi/c_lo split keeps the affine constant to
fp32 accuracy. The K=2^7 scale keeps fp16 outputs away from the
denormal/FTZ zone without overflowing genuine candidates.

Per (b, face-tile) iteration, [128, 1024] ops balanced across engines
(Act / DVE / Pool(gpsimd) ~2.6us each, steady II 2.66us):
  a  = relu(s_ab)            Act   (makes the max-chain >= 0)
  m1 = max(a, s_ca)          DVE   (one PSUM operand per vector op)
  m2 = max(m1, s_bc) -> f16  DVE
  sf = h^2 -> f16            Act
  qm = m2^2 -> f16           Act cols [0,CQ) / Pool f16 mult [CQ,1024)
  S  = qm + sf   f16         DVE cols [0,CSOL) / Pool add [CSOL,1024)
PSUM: 2 names x bufs=2 x 2 banks double-buffer matmuls against drains.
Startup: 3 junk bf16 matmuls ramp the PE p-state while the weights DMA
in three parallel chunks (SP head slice / Pool points / Act bulk).
"""

import numpy as np

B, G, V, F = 2, 1024, 6890, 13776
NCORES = 8
FC = F // NCORES            # 1722 faces per core
FTILES = 14                 # ceil(1722/128)
FPAD = FTILES * 128         # 1792
GCHUNK = 512
NQ = 4                      # quantities: s_ab, s_ca, s_bc, h
NROW = 5                    # moving rows: px, py, pz, 1, 1
PCOLS = B * G               # snapped points packed at the head of wq
WCOLS = B * FTILES * NQ * 128
SC = np.float64(128.0)      # weight scale (exact power of 2)
BIGH = np.float32(1e6)      # h-const for pad/degenerate faces -> S = inf
TOPM = 48                   # host: faces re-ranked exactly per point

_CACHE = {}


def _build_bass():
    import concourse.bass as bass
    import concourse.bacc as bacc
    import concourse.mybir as mybir
    from concourse.tile import TileContext

    dt = mybir.dt.float32
    dtr = mybir.dt.float32r
    dth = mybir.dt.float16
    Alu = mybir.AluOpType
    Act = mybir.ActivationFunctionType

    nc = bacc.Bacc()

    wq_d = nc.declare_dram_parameter("wq", [NROW, PCOLS + WCOLS], dtr,
                                     isOutput=False)
    outs_d = nc.declare_dram_parameter("out_s", [B, FTILES, 128, G], dth,
                                       isOutput=True)

    Vv = nc.vector
    Gg = nc.gpsimd
    Ss = nc.scalar
    Tt = nc.tensor
    Sy = nc.sync

    GW = 2 * GCHUNK
    sh = [128, GW]

    with TileContext(nc) as tc:
        with (
            tc.tile_pool(name="cpool", bufs=1) as cpool,
            tc.tile_pool(name="work", bufs=1) as work,
            tc.tile_pool(name="mm", bufs=2, space="PSUM") as mm,
        ):
            wq_s = cpool.tile([NROW, PCOLS + WCOLS], dtr, name="wq_s")
            # One head DMA delivers the points block plus the first three
            # face-tiles' weights so compute starts while the bulk
            # transfers through the Act DGE in parallel.
            HDW = PCOLS + 3 * NQ * 128
            Sy.dma_start(wq_s[:, :HDW], wq_d[:, :HDW])
            Ss.dma_start(wq_s[:, HDW:], wq_d[:, HDW:])
            # Warm up the PE p-state during the DMA wait with junk matmuls
            # on a zeroed stationary tile (results never read).
            dtb = mybir.dt.bfloat16
            wz = cpool.tile([NROW, 128 + GCHUNK], dtb, name="wz")
            Gg.memset(wz[:], 0)
            warm = mm.tile(sh, dt, name="px")
            for _ in range(3):
                Tt.matmul(warm[:, :GCHUNK], wz[:, :128], wz[:, 128:],
                          start=True, stop=True)

            def wt(nm, dtype=dth):
                return work.tile(sh, dtype, name=nm, bufs=4)

            def MM(b, ft, q, name):
                t = mm.tile(sh, dt, name=name)
                col = PCOLS + ((b * FTILES + ft) * NQ + q) * 128
                for h in range(2):
                    g0 = b * G + h * GCHUNK
                    Tt.matmul(t[:, h * GCHUNK:(h + 1) * GCHUNK],
                              wq_s[:, col:col + 128],
                              wq_s[:, g0:g0 + GCHUNK],
                              start=True, stop=True)
                return t

            # S = h^2 + relu(max(s_ab, s_ca, s_bc))^2. a = relu(s_ab)
            # makes the max-chain >= 0 so the square needs no relu.
            # Constraints: GPSIMD (Pool) can't touch PSUM and only does
            # add/mult; vector ops read at most one PSUM operand. Engine
            # balance: Act {a, sf, part of qm}, DVE {m1, m2->f16, part of
            # So}, Pool {rest of qm as f16 mult, rest of So}. PSUM: 2
            # names x bufs=2 (8 banks) double-buffers matmuls vs drains.
            CQ = 464                # qm columns squared by Act
            CSOL = 400              # So columns added by DVE

            def stage2(c):
                """Square + add + DMA for a finished face-tile."""
                b, ft, m2, sf = c
                qm = wt("qm")
                Ss.activation(qm[:, :CQ], m2[:, :CQ], Act.Square)
                Gg.tensor_tensor(qm[:, CQ:], m2[:, CQ:], m2[:, CQ:],
                                 Alu.mult)
                So = wt("So")
                last = (b, ft) == (B - 1, FTILES - 1)
                cso = GW if last else CSOL
                Vv.tensor_tensor(So[:, :cso], qm[:, :cso],
                                 sf[:, :cso], Alu.add)
                if not last:
                    Gg.tensor_tensor(So[:, CSOL:], qm[:, CSOL:],
                                     sf[:, CSOL:], Alu.add)
                if last:
                    # tail: overlap the DMA issue with the Pool add
                    Sy.dma_start(outs_d[b, ft, :, :CSOL], So[:, :CSOL])
                    Sy.dma_start(outs_d[b, ft, :, CSOL:], So[:, CSOL:])
                else:
                    Sy.dma_start(outs_d[b, ft], So[:])

            for b in range(B):
                for ft in range(FTILES):
                    pab = MM(b, ft, 0, "px")
                    pca = MM(b, ft, 1, "py")
                    a = wt("a", dt)
                    Ss.activation(a[:], pab[:], Act.Relu)
                    m1 = wt("m1", dt)
                    Vv.tensor_tensor(m1[:], a[:], pca[:], Alu.max)
                    pbc = MM(b, ft, 2, "px")
                    ph_ = MM(b, ft, 3, "py")
                    m2 = wt("m2")
                    Vv.tensor_tensor(m2[:], m1[:], pbc[:], Alu.max)
                    sf = wt("sf")
                    Ss.activation(sf[:], ph_[:], Act.Square)
                    stage2((b, ft, m2, sf))
    nc.finalize()
    return nc


def _get_nc():
    if "nc" not in _CACHE:
        _CACHE["nc"] = _build_bass()
    return _CACHE["nc"]


def _round_fp32r(x):
    """Round fp32 -> fp32r container (11-bit mantissa, RNE)."""
    u = np.ascontiguousarray(x, np.float32).view(np.uint32)
    base = u & np.uint32(0xFFFFF000)
    low = u & np.uint32(0x00000FFF)
    half = np.uint32(0x800)
    lsb = (base >> np.uint32(12)) & np.uint32(1)
    up = (low > half) | ((low == half) & (lsb == 1))
    return np.where(up, base + np.uint32(0x1000), base).view(np.float32)


def _face_geom(batch_body_verts, body_faces):
    """Snapped fp64 per-face geometry + degeneracy mask, per batch.

    Returns (a, b, c [B,F,3] fp64 snapped verts, degen [B,F] bool)."""
    f32 = np.float32
    out = []
    for bi in range(B):
        vs = _round_fp32r(batch_body_verts[bi].astype(f32)).astype(np.float64)
        fv = vs[body_faces]
        a, bb, cc = fv[:, 0], fv[:, 1], fv[:, 2]
        ab, ac = bb - a, cc - a
        n = np.cross(ab, ac)
        nn = np.linalg.norm(n, axis=1)
        den = (np.sum(ab * ab, -1) * np.sum(ac * ac, -1)
               - np.sum(ab * ac, -1) ** 2)
        degen = (den < 1e-8) | (nn < 1e-10)
        out.append((a, bb, cc, n, nn, degen))
    return out


def _core_inputs(batch_garment_verts, batch_body_verts, body_faces, geom):
    f32 = np.float32
    gv = batch_garment_verts.astype(f32)
    p_snap = _round_fp32r(gv)                             # [B,G,3]
    p5 = np.zeros((NROW, B * G), f32)
    for bi in range(B):
        p5[0:3, bi * G:(bi + 1) * G] = p_snap[bi].T
    p5[3] = 1.0
    p5[4] = 1.0

    def hi_lo(c):
        chi = _round_fp32r(c.astype(f32)).astype(np.float64)
        clo = _round_fp32r((c - chi).astype(f32))
        return chi.astype(f32), clo

    in_maps = []
    for c in range(NCORES):
        sl = slice(c * FC, (c + 1) * FC)
        wq = np.zeros((NROW, PCOLS + WCOLS), f32)
        wq[:, :PCOLS] = p5
        for bi in range(B):
            a, bb, cc, n, nn, degen = geom[bi]
            a, bb, cc = a[sl], bb[sl], cc[sl]
            n, nn, degen = n[sl], nn[sl], degen[sl]
            nh = n / np.maximum(nn, 1e-30)[:, None]
            rows = np.zeros((NQ, NROW, FPAD), f32)
            for qi, (eu, ev, w) in enumerate(
                    ((a, bb, cc), (cc, a, bb), (bb, cc, a))):
                ed = ev - eu
                m = np.cross(ed, nh)
                m = m / np.maximum(np.linalg.norm(m, axis=1), 1e-30)[:, None]
                flip = np.sum(m * (w - eu), -1) > 0
                m = np.where(flip[:, None], -m, m) * SC
                m_r = _round_fp32r(m.astype(f32)).astype(np.float64)
                chi, clo = hi_lo(-np.sum(m_r * eu, -1))
                rows[qi, 0:3, :FC] = m_r.T.astype(f32)
                rows[qi, 3, :FC] = chi
                rows[qi, 4, :FC] = clo
            nh_r = _round_fp32r((nh * SC).astype(f32)).astype(np.float64)
            chi, clo = hi_lo(-np.sum(nh_r * a, -1))
            rows[3, 0:3, :FC] = nh_r.T.astype(f32)
            rows[3, 3, :FC] = np.where(degen, BIGH, chi)
            rows[3, 4, :FC] = np.where(degen, 0.0, clo)
            rows[3, 0:3, :FC][:, degen] = 0.0
            rows[0:3, :, :FC][:, :, degen] = 0.0
            rows[3, 3, FC:] = BIGH                        # pad faces -> inf
            for ft in range(FTILES):
                fsl = slice(ft * 128, (ft + 1) * 128)
                for q in range(NQ):
                    c0 = PCOLS + ((bi * FTILES + ft) * NQ + q) * 128
                    wq[:, c0:c0 + 128] = rows[q][:, fsl]
        in_maps.append({"wq": wq})
    return in_maps


def _d2_exact64_cand(p, bverts, faces, cand):
    """Exact fp64 point-triangle dist^2 for candidate faces. cand [G,C]."""
    fv = bverts[faces[cand]].astype(np.float64)      # [G,C,3,3]
    a, b, c = fv[:, :, 0], fv[:, :, 1], fv[:, :, 2]
    q = p.astype(np.float64)[:, None, :]
    best = np.full(cand.shape, np.inf)
    for ea, eb in ((a, b), (b, c), (c, a)):
        ed = eb - ea
        L2 = np.sum(ed * ed, -1)
        pe = q - ea
        t = np.clip(np.sum(pe * ed, -1) / np.maximum(L2, 1e-300), 0, 1)
        d = pe - t[..., None] * ed
        best = np.minimum(best, np.sum(d * d, -1))
    ab, ac = b - a, c - a
    n = np.cross(ab, ac)
    naa = np.sum(ab * ab, -1); nab = np.sum(ab * ac, -1)
    ncc = np.sum(ac * ac, -1)
    den = naa * ncc - nab * nab
    pa = q - a
    d1 = np.sum(pa * ab, -1); d2_ = np.sum(pa * ac, -1)
    vb = ncc * d1 - nab * d2_; vc = naa * d2_ - nab * d1
    va = den - vb - vc
    inside = (vb >= 0) & (vc >= 0) & (va >= 0) & (den > 1e-300)
    hn = np.sum(pa * n, -1)
    h2 = hn * hn / np.maximum(den, 1e-300)
    return np.where(inside, np.minimum(best, h2), best)


def _host_finish(g_verts, b_verts, faces, tri):
    """Exact reference finish for the winning face of each garment point."""
    f32 = np.float32
    EPS = f32(1e-10)

    def safe(x):
        return np.where(np.abs(x) < 1e-12, f32(1e-12), x).astype(f32)

    fverts = b_verts[faces]
    a_, b_, c_ = fverts[:, 0], fverts[:, 1], fverts[:, 2]
    fn_raw = np.cross(b_ - a_, c_ - a_).astype(f32)
    vn = np.zeros_like(b_verts)
    for k in range(3):
        np.add.at(vn, faces[:, k], fn_raw)
    vn = vn / (np.linalg.norm(vn, axis=-1, keepdims=True).astype(f32) + EPS)
    fn = fn_raw / (np.linalg.norm(fn_raw, axis=-1, keepdims=True).astype(f32) + EPS)

    a = a_[tri]; bb = b_[tri]; cc = c_[tri]
    q = g_verts
    ab = bb - a; ac = cc - a
    ap = q - a
    d1 = np.sum(ab * ap, -1); d2 = np.sum(ac * ap, -1)
    bp = q - bb
    d3 = np.sum(ab * bp, -1); d4 = np.sum(ac * bp, -1)
    cp = q - cc
    d5 = np.sum(ab * cp, -1); d6 = np.sum(ac * cp, -1)
    vc = d1 * d4 - d3 * d2
    vb = d5 * d2 - d1 * d6
    va = d3 * d6 - d5 * d4
    denom = safe(va + vb + vc)
    v, w = (vb / denom).astype(f32), (vc / denom).astype(f32)
    part = np.zeros(v.shape, np.int32)
    t_bc = ((d4 - d3) / safe((d4 - d3) + (d5 - d6))).astype(f32)
    m = (va <= 0) & (d4 - d3 >= 0) & (d5 - d6 >= 0)
    v = np.where(m, 1.0 - t_bc, v).astype(f32)
    w = np.where(m, t_bc, w).astype(f32)
    part = np.where(m, 2, part)
    t_ac = (d2 / safe(d2 - d6)).astype(f32)
    m = (vb <= 0) & (d2 >= 0) & (d6 <= 0)
    v = np.where(m, 0.0, v).astype(f32)
    w = np.where(m, t_ac, w).astype(f32)
    part = np.where(m, 3, part)
    m = (d6 >= 0) & (d5 <= d6)
    v = np.where(m, 0.0, v).astype(f32)
    w = np.where(m, 1.0, w).astype(f32)
    part = np.where(m, 6, part)
    t_ab = (d1 / safe(d1 - d3)).astype(f32)
    m = (vc <= 0) & (d1 >= 0) & (d3 <= 0)
    v = np.where(m, t_ab, v).astype(f32)
    w = np.where(m, 0.0, w).astype(f32)
    part = np.where(m, 1, part)
    m = (d3 >= 0) & (d4 <= d3)
    v = np.where(m, 1.0, v).astype(f32)
    w = np.where(m, 0.0, w).astype(f32)
    part = np.where(m, 5, part)
    m = (d1 <= 0) & (d2 <= 0)
    v = np.where(m, 0.0, v).astype(f32)
    w = np.where(m, 0.0, w).astype(f32)
    part = np.where(m, 4, part)
    npt = a + v[:, None] * ab + w[:, None] * ac

    fidx = faces[tri]
    gar = np.arange(len(tri))
    take = lambda col: vn[fidx[gar, col]]
    n_face = fn[tri]
    n_vert = take(np.clip(part - 4, 0, 2))
    n_edge = take(np.clip(part - 1, 0, 2)) + take(np.mod(part, 3))
    n = np.where((part == 0)[:, None], n_face,
                 np.where((part > 3)[:, None], n_vert, n_edge)).astype(f32)
    n = n / (np.linalg.norm(n, axis=-1, keepdims=True).astype(f32) + EPS)
    return np.sum((g_verts - npt) * n, axis=1).astype(f32)


def kernel(batch_garment_verts, batch_body_verts, body_faces, _profile=None):
    from concourse.bass_utils import run_bass_kernel_spmd

    batch_garment_verts = np.asarray(batch_garment_verts, dtype=np.float32)
    batch_body_verts = np.asarray(batch_body_verts, dtype=np.float32)
    body_faces = np.asarray(body_faces)

    nc = _get_nc()
    geom = _face_geom(batch_body_verts, body_faces)
    in_maps = _core_inputs(batch_garment_verts, batch_body_verts,
                           body_faces, geom)
    kwargs = dict(_profile) if _profile else {}
    res = run_bass_kernel_spmd(nc, in_maps, list(range(NCORES)), **kwargs)
    if _profile is not None:
        _CACHE["last_results"] = res

    vals = np.stack([np.asarray(r["out_s"]) for r in res.results])
    # [NC,B,FT,128,G] -> [B,G,NC*FPAD]
    flat = vals.astype(np.float32).transpose(1, 4, 0, 2, 3).reshape(
        B, G, NCORES * FPAD)
    local = np.arange(NCORES * FPAD) % FPAD
    flat = np.where(local[None, None, :] < FC, flat, np.inf)
    out = np.empty((B, G), np.float32)
    for b in range(B):
        degen_ids = np.nonzero(geom[b][5])[0]
        top = np.argpartition(flat[b], TOPM, axis=1)[:, :TOPM]  # [G, M]
        cand = (top // FPAD) * FC + (top % FPAD)                # global face id
        if degen_ids.size:
            cand = np.concatenate(
                [cand, np.broadcast_to(degen_ids, (G, degen_ids.size))], 1)
        dref = _d2_exact64_cand(batch_garment_verts[b], batch_body_verts[b],
                                body_faces, cand)
        mn = dref.min(axis=1, keepdims=True)
        sel = np.where(dref == mn, cand, F + 1)
        tri = sel.min(axis=1)
        out[b] = _host_finish(batch_garment_verts[b], batch_body_verts[b],
                              body_faces, tri)
    return out
